# revision 10
# baseline (speedup 1.0000x reference)
"""Trainium2 Bass kernel for nn_DiscAdvLossForTarget_min (v9).

Math: loss = (1/B) * sum_b V_b/T_b with a = exp(x - e), w = log1p(a),
V = sum_i a*w, T = sum_i a.

The per-row reduction V moves off the DVE onto the (otherwise idle)
tensor engine: scaling rows by 1/T_b turns sum_b V_b/T_b into a full
sum, which PE does as a matmul with the per-block invT column as the
stationary weight vector:

  psum[f] += sum_p invT[p] * pw[p, f],   pw = a * w
  w = (bits(1+a) - K0) * S               (bit-log log1p, DVE 4x)

Blocks are processed in GROUPS (PLAN, mostly 4 blocks) so that
 - each group's input DMA is one dma_start with 16KB contiguous
   per-partition lines (row = p*64 + n layout), which is what keeps the
   aggregate DMA at ~330 GB/s (v8's per-block 4KB lines cost +13us);
 - the three big DVE ops (a+1, bit-log, a*w) each run once per group at
   FD=4000, amortizing the ~120ns per-op SBUF bubble 4x;
 - PE matmuls come in bursts of 8, keeping the PE p-state warmer.
Groups taper to 1-2 blocks at both ends: short first-DMA latency at
start, short exp->mm drain at the end.

Engine budget/core: DMA ~100us (32.8MB at ~330GB/s = the floor),
ACT ~89us (64 exp+accum, nothing else), DVE ~78us, PE ~60us.

Host: loss = (sum of per-class psums over cores) / B.
"""

import numpy as np

import concourse.bacc as bacc
import concourse.bass as bass
import concourse.tile as tile
from concourse import bass_utils, mybir

N_CORES = 8
B_FULL = 65536
C1 = 1001
C = 1000
P = 128
B_SHARD = B_FULL // N_CORES  # 8192
N_BLOCKS = B_SHARD // P  # 64
G_MAX = 4

# bit-log fit: w ~= (bits(y) - K0) * S, a-weighted LS vs log1p
BITLOG_S = 0.00541268
BITLOG_K0 = 16248.447

HALF = 500  # psum bank holds 512 fp32; split the 1000 classes in two

PLAN = [1, 1, 2] + [4] * 13 + [2, 2, 2, 1, 1]
assert sum(PLAN) == N_BLOCKS

_nc_cache = None


def _build() -> bass.Bass:
    global _nc_cache
    if _nc_cache is not None:
        return _nc_cache

    nc = bacc.Bacc("TRN2", debug=False)
    x = nc.dram_tensor("x", [B_SHARD, C1], mybir.dt.float32, kind="ExternalInput").ap()
    o = nc.dram_tensor("o", [1, C], mybir.dt.float32, kind="ExternalOutput").ap()

    # row = p*64 + n: group DMAs get gsz*4004B contiguous per partition
    x_r = x.rearrange("(p n) m -> p n m", p=P, n=N_BLOCKS)

    with tile.TileContext(nc) as tc:
        with (
            tc.tile_pool(name="xin", bufs=5) as xin,
            tc.tile_pool(name="apool", bufs=5) as apool,
            tc.tile_pool(name="ypool", bufs=2) as ypool,
            tc.tile_pool(name="wpool", bufs=2) as wpool,
            tc.tile_pool(name="pwpool", bufs=3) as pwpool,
            tc.tile_pool(name="nep", bufs=4) as nep,
            tc.tile_pool(name="accp", bufs=1) as accp,
            tc.tile_pool(name="psp", bufs=1, space="PSUM") as psp,
        ):
            T = accp.tile([P, N_BLOCKS], mybir.dt.float32)
            iT = accp.tile([P, N_BLOCKS], mybir.dt.bfloat16)
            out_sb = accp.tile([1, C], mybir.dt.float32)
            ps0 = psp.tile([1, HALF], mybir.dt.float32)
            ps1 = psp.tile([1, C - HALF], mybir.dt.float32)

            # dma_starts are software-pipelined two groups ahead; the exp
            # bias (-e) is computed on the otherwise-idle GPSIMD engine so
            # ACT never waits behind the DVE's big streaming ops for it.
            starts = []
            n0 = 0
            for gsz in PLAN:
                starts.append(n0)
                n0 += gsz

            # tiny dummy exp: walrus puts the ACT_TABLE_LOAD right before
            # the first activation, so this hoists the ~2.7us table load
            # into the NEFF preamble window instead of serializing it with
            # the first real exp.
            dummy = accp.tile([1, 2], mybir.dt.float32)
            dummy_o = accp.tile([1, 2], mybir.dt.bfloat16)
            nc.any.memset(dummy, 0.0)
            nc.scalar.activation(
                out=dummy_o, in_=dummy,
                func=mybir.ActivationFunctionType.Exp, scale=1.0,
            )

            def issue_load(g):
                gsz, n0 = PLAN[g], starts[g]
                xt = xin.tile([P, G_MAX, C1], mybir.dt.float32, tag="xt")
                nc.sync.dma_start(
                    out=xt[:, 0:gsz, :], in_=x_r[:, n0 : n0 + gsz, :]
                )
                neg_e = nep.tile([P, G_MAX], mybir.dt.float32, tag="ne")
                # DVE is idle before the pipeline fills; after that keep
                # the bias op off the busy DVE stream (idle GPSIMD instead)
                eng = nc.vector if g < 2 else nc.gpsimd
                eng.tensor_scalar_mul(
                    neg_e[:, 0:gsz], xt[:, 0:gsz, C], -1.0
                )
                return xt, neg_e

            DEPTH = 3
            window = [issue_load(g) for g in range(DEPTH)]
            for g, gsz in enumerate(PLAN):
                n0 = starts[g]
                xt, neg_e = window.pop(0)
                if g + DEPTH < len(PLAN):
                    window.append(issue_load(g + DEPTH))

                aa = apool.tile([P, G_MAX, C], mybir.dt.bfloat16, tag="aa")
                for j in range(gsz):
                    n = n0 + j
                    nc.scalar.activation(
                        out=aa[:, j, :],
                        in_=xt[:, j, 0:C],
                        func=mybir.ActivationFunctionType.Exp,
                        bias=neg_e[:, j : j + 1],
                        scale=1.0,
                        accum_out=T[:, n : n + 1],
                    )

                aa_f = aa[:, 0:gsz, :].rearrange("p g c -> p (g c)")
                yy = ypool.tile([P, G_MAX, C], mybir.dt.bfloat16, tag="yy")
                yy_f = yy[:, 0:gsz, :].rearrange("p g c -> p (g c)")
                nc.vector.tensor_scalar_add(yy_f, aa_f, 1.0)

                with nc.allow_low_precision(reason="bf16 1/T weights; error averages out over 64k rows"):
                    nc.vector.reciprocal(
                        iT[:, n0 : n0 + gsz], T[:, n0 : n0 + gsz]
                    )

                ww = wpool.tile([P, G_MAX, C], mybir.dt.bfloat16, tag="ww")
                ww_f = ww[:, 0:gsz, :].rearrange("p g c -> p (g c)")
                nc.vector.tensor_scalar(
                    out=ww_f,
                    in0=yy_f.bitcast(mybir.dt.uint16),
                    scalar1=BITLOG_K0,
                    scalar2=BITLOG_S,
                    op0=mybir.AluOpType.subtract,
                    op1=mybir.AluOpType.mult,
                )

                pw = pwpool.tile([P, G_MAX, C], mybir.dt.bfloat16, tag="pw")
                pw_f = pw[:, 0:gsz, :].rearrange("p g c -> p (g c)")
                nc.vector.tensor_tensor(
                    out=pw_f, in0=aa_f, in1=ww_f, op=mybir.AluOpType.mult
                )

                for j in range(gsz):
                    n = n0 + j
                    first, last = n == 0, n == N_BLOCKS - 1
                    nc.tensor.matmul(
                        ps0, iT[:, n : n + 1], pw[:, j, 0:HALF],
                        start=first, stop=last,
                    )
                    nc.tensor.matmul(
                        ps1, iT[:, n : n + 1], pw[:, j, HALF:C],
                        start=first, stop=last,
                    )

            nc.scalar.copy(out_sb[:, 0:HALF], ps0)
            nc.vector.tensor_copy(out_sb[:, HALF:C], ps1)
            nc.sync.dma_start(out=o, in_=out_sb)

    nc.finalize()
    _nc_cache = nc
    return nc


LAST_RESULTS = None


def kernel(input: np.ndarray, target: np.ndarray | None = None, _trace: bool = False, **_unused) -> np.ndarray:
    global LAST_RESULTS
    input = np.ascontiguousarray(np.asarray(input, dtype=np.float32))
    assert input.shape == (B_FULL, C1), input.shape

    nc = _build()
    in_maps = [
        {"x": input[i * B_SHARD : (i + 1) * B_SHARD]} for i in range(N_CORES)
    ]
    res = bass_utils.run_bass_kernel_spmd(
        nc, in_maps, core_ids=list(range(N_CORES)), trace=_trace
    )
    LAST_RESULTS = res
    total = np.float64(0.0)
    for r in res.results:
        total += np.asarray(r["o"], dtype=np.float64).sum()
    loss = total / B_FULL
    return np.float32(loss)


# revision 11
# speedup vs baseline: 1.0253x; 1.0253x over previous
"""Trainium2 Bass kernel for nn_DiscAdvLossForTarget_min (v9).

Math: loss = (1/B) * sum_b V_b/T_b with a = exp(x - e), w = log1p(a),
V = sum_i a*w, T = sum_i a.

The per-row reduction V moves off the DVE onto the (otherwise idle)
tensor engine: scaling rows by 1/T_b turns sum_b V_b/T_b into a full
sum, which PE does as a matmul with the per-block invT column as the
stationary weight vector:

  psum[f] += sum_p invT[p] * pw[p, f],   pw = a * w
  w = (bits(1+a) - K0) * S               (bit-log log1p, DVE 4x)

Blocks are processed in GROUPS (PLAN, mostly 4 blocks) so that
 - each group's input DMA is one dma_start with 16KB contiguous
   per-partition lines (row = p*64 + n layout), which is what keeps the
   aggregate DMA at ~330 GB/s (v8's per-block 4KB lines cost +13us);
 - the three big DVE ops (a+1, bit-log, a*w) each run once per group at
   FD=4000, amortizing the ~120ns per-op SBUF bubble 4x;
 - PE matmuls come in bursts of 8, keeping the PE p-state warmer.
Groups taper to 1-2 blocks at both ends: short first-DMA latency at
start, short exp->mm drain at the end.

Engine budget/core: DMA ~100us (32.8MB at ~330GB/s = the floor),
ACT ~89us (64 exp+accum, nothing else), DVE ~78us, PE ~60us.

Host: loss = (sum of per-class psums over cores) / B.
"""

import numpy as np

import concourse.bacc as bacc
import concourse.bass as bass
import concourse.tile as tile
from concourse import bass_utils, mybir

N_CORES = 8
B_FULL = 65536
C1 = 1001
C = 1000
P = 128
B_SHARD = B_FULL // N_CORES  # 8192
N_BLOCKS = B_SHARD // P  # 64
G_MAX = 4

# bit-log fit: w ~= (bits(y) - K0) * S, a-weighted LS vs log1p
BITLOG_S = 0.00541268
BITLOG_K0 = 16248.447

HALF = 500  # psum bank holds 512 fp32; split the 1000 classes in two

PLAN = [1, 1, 2] + [4] * 13 + [2, 2, 2, 1, 1]
assert sum(PLAN) == N_BLOCKS

_nc_cache = None


def _build() -> bass.Bass:
    global _nc_cache
    if _nc_cache is not None:
        return _nc_cache

    nc = bacc.Bacc("TRN2", debug=False)
    x = nc.dram_tensor("x", [B_SHARD, C1], mybir.dt.float32, kind="ExternalInput").ap()
    o = nc.dram_tensor("o", [1, C], mybir.dt.float32, kind="ExternalOutput").ap()

    # row = p*64 + n: group DMAs get gsz*4004B contiguous per partition
    x_r = x.rearrange("(p n) m -> p n m", p=P, n=N_BLOCKS)

    with tile.TileContext(nc) as tc:
        with (
            tc.tile_pool(name="xin", bufs=5) as xin,
            tc.tile_pool(name="apool", bufs=5) as apool,
            tc.tile_pool(name="ypool", bufs=2) as ypool,
            tc.tile_pool(name="wpool", bufs=2) as wpool,
            tc.tile_pool(name="pwpool", bufs=3) as pwpool,
            tc.tile_pool(name="nep", bufs=4) as nep,
            tc.tile_pool(name="accp", bufs=1) as accp,
            tc.tile_pool(name="psp", bufs=1, space="PSUM") as psp,
        ):
            T = accp.tile([P, N_BLOCKS], mybir.dt.float32)
            iT = accp.tile([P, N_BLOCKS], mybir.dt.bfloat16)
            out_sb = accp.tile([1, C], mybir.dt.float32)
            ps0 = psp.tile([1, HALF], mybir.dt.float32)
            ps1 = psp.tile([1, C - HALF], mybir.dt.float32)

            # dma_starts are software-pipelined two groups ahead; the exp
            # bias (-e) is computed on the otherwise-idle GPSIMD engine so
            # ACT never waits behind the DVE's big streaming ops for it.
            starts = []
            n0 = 0
            for gsz in PLAN:
                starts.append(n0)
                n0 += gsz

            def issue_load(g):
                gsz, n0 = PLAN[g], starts[g]
                xt = xin.tile([P, G_MAX, C1], mybir.dt.float32, tag="xt")
                nc.sync.dma_start(
                    out=xt[:, 0:gsz, :], in_=x_r[:, n0 : n0 + gsz, :]
                )
                neg_e = nep.tile([P, G_MAX], mybir.dt.float32, tag="ne")
                nc.gpsimd.tensor_scalar_mul(
                    neg_e[:, 0:gsz], xt[:, 0:gsz, C], -1.0
                )
                return xt, neg_e

            DEPTH = 3
            window = [issue_load(g) for g in range(DEPTH)]
            for g, gsz in enumerate(PLAN):
                n0 = starts[g]
                xt, neg_e = window.pop(0)
                if g + DEPTH < len(PLAN):
                    window.append(issue_load(g + DEPTH))

                aa = apool.tile([P, G_MAX, C], mybir.dt.bfloat16, tag="aa")
                for j in range(gsz):
                    n = n0 + j
                    nc.scalar.activation(
                        out=aa[:, j, :],
                        in_=xt[:, j, 0:C],
                        func=mybir.ActivationFunctionType.Exp,
                        bias=neg_e[:, j : j + 1],
                        scale=1.0,
                        accum_out=T[:, n : n + 1],
                    )

                aa_f = aa[:, 0:gsz, :].rearrange("p g c -> p (g c)")
                yy = ypool.tile([P, G_MAX, C], mybir.dt.bfloat16, tag="yy")
                yy_f = yy[:, 0:gsz, :].rearrange("p g c -> p (g c)")
                nc.vector.tensor_scalar_add(yy_f, aa_f, 1.0)

                with nc.allow_low_precision(reason="bf16 1/T weights; error averages out over 64k rows"):
                    nc.vector.reciprocal(
                        iT[:, n0 : n0 + gsz], T[:, n0 : n0 + gsz]
                    )

                ww = wpool.tile([P, G_MAX, C], mybir.dt.bfloat16, tag="ww")
                ww_f = ww[:, 0:gsz, :].rearrange("p g c -> p (g c)")
                nc.vector.tensor_scalar(
                    out=ww_f,
                    in0=yy_f.bitcast(mybir.dt.uint16),
                    scalar1=BITLOG_K0,
                    scalar2=BITLOG_S,
                    op0=mybir.AluOpType.subtract,
                    op1=mybir.AluOpType.mult,
                )

                pw = pwpool.tile([P, G_MAX, C], mybir.dt.bfloat16, tag="pw")
                pw_f = pw[:, 0:gsz, :].rearrange("p g c -> p (g c)")
                nc.vector.tensor_tensor(
                    out=pw_f, in0=aa_f, in1=ww_f, op=mybir.AluOpType.mult
                )

                for j in range(gsz):
                    n = n0 + j
                    first, last = n == 0, n == N_BLOCKS - 1
                    nc.tensor.matmul(
                        ps0, iT[:, n : n + 1], pw[:, j, 0:HALF],
                        start=first, stop=last,
                    )
                    nc.tensor.matmul(
                        ps1, iT[:, n : n + 1], pw[:, j, HALF:C],
                        start=first, stop=last,
                    )

            nc.scalar.copy(out_sb[:, 0:HALF], ps0)
            nc.vector.tensor_copy(out_sb[:, HALF:C], ps1)
            nc.sync.dma_start(out=o, in_=out_sb)

    nc.finalize()
    _nc_cache = nc
    return nc


LAST_RESULTS = None


def kernel(input: np.ndarray, target: np.ndarray | None = None, _trace: bool = False, **_unused) -> np.ndarray:
    global LAST_RESULTS
    input = np.ascontiguousarray(np.asarray(input, dtype=np.float32))
    assert input.shape == (B_FULL, C1), input.shape

    nc = _build()
    in_maps = [
        {"x": input[i * B_SHARD : (i + 1) * B_SHARD]} for i in range(N_CORES)
    ]
    res = bass_utils.run_bass_kernel_spmd(
        nc, in_maps, core_ids=list(range(N_CORES)), trace=_trace
    )
    LAST_RESULTS = res
    total = np.float64(0.0)
    for r in res.results:
        total += np.asarray(r["o"], dtype=np.float64).sum()
    loss = total / B_FULL
    return np.float32(loss)


# revision 12
# speedup vs baseline: 1.1321x; 1.1042x over previous
"""Trainium2 Bass kernel for nn_DiscAdvLossForTarget_min (v9).

Math: loss = (1/B) * sum_b V_b/T_b with a = exp(x - e), w = log1p(a),
V = sum_i a*w, T = sum_i a.

The per-row reduction V moves off the DVE onto the (otherwise idle)
tensor engine: scaling rows by 1/T_b turns sum_b V_b/T_b into a full
sum, which PE does as a matmul with the per-block invT column as the
stationary weight vector:

  psum[f] += sum_p invT[p] * pw[p, f],   pw = a * w
  w = (bits(1+a) - K0) * S               (bit-log log1p, DVE 4x)

Blocks are processed in GROUPS (PLAN, mostly 4 blocks) so that
 - each group's input DMA is one dma_start with 16KB contiguous
   per-partition lines (row = p*64 + n layout), which is what keeps the
   aggregate DMA at ~330 GB/s (v8's per-block 4KB lines cost +13us);
 - the three big DVE ops (a+1, bit-log, a*w) each run once per group at
   FD=4000, amortizing the ~120ns per-op SBUF bubble 4x;
 - PE matmuls come in bursts of 8, keeping the PE p-state warmer.
Groups taper to 1-2 blocks at both ends: short first-DMA latency at
start, short exp->mm drain at the end.

Engine budget/core: DMA ~100us (32.8MB at ~330GB/s = the floor),
ACT ~89us (64 exp+accum, nothing else), DVE ~78us, PE ~60us.

Host: loss = (sum of per-class psums over cores) / B.
"""

import numpy as np

import concourse.bacc as bacc
import concourse.bass as bass
import concourse.tile as tile
from concourse import bass_utils, mybir

N_CORES = 8
B_FULL = 65536
C1 = 1001
C = 1000
P = 128
B_SHARD = B_FULL // N_CORES  # 8192
N_BLOCKS = B_SHARD // P  # 64
G_MAX = 4

# bit-log fit: w ~= (bits(y) - K0) * S, a-weighted LS vs log1p
BITLOG_S = 0.00541268
BITLOG_K0 = 16248.447

HALF = 500  # psum bank holds 512 fp32; split the 1000 classes in two

PLAN = [1, 1, 2] + [4] * 14 + [2, 1, 1]
assert sum(PLAN) == N_BLOCKS

_nc_cache = None


def _build() -> bass.Bass:
    global _nc_cache
    if _nc_cache is not None:
        return _nc_cache

    nc = bacc.Bacc("TRN2", debug=False)
    x = nc.dram_tensor("x", [B_SHARD, C1], mybir.dt.float32, kind="ExternalInput").ap()
    o = nc.dram_tensor("o", [1, C], mybir.dt.float32, kind="ExternalOutput").ap()

    # row = p*64 + n: group DMAs get gsz*4004B contiguous per partition
    x_r = x.rearrange("(p n) m -> p n m", p=P, n=N_BLOCKS)

    with tile.TileContext(nc) as tc:
        with (
            tc.tile_pool(name="xin", bufs=5) as xin,
            tc.tile_pool(name="apool", bufs=5) as apool,
            tc.tile_pool(name="ypool", bufs=2) as ypool,
            tc.tile_pool(name="wpool", bufs=2) as wpool,
            tc.tile_pool(name="pwpool", bufs=3) as pwpool,
            tc.tile_pool(name="nep", bufs=4) as nep,
            tc.tile_pool(name="accp", bufs=1) as accp,
            tc.tile_pool(name="psp", bufs=1, space="PSUM") as psp,
        ):
            T = accp.tile([P, N_BLOCKS], mybir.dt.float32)
            iT = accp.tile([P, N_BLOCKS], mybir.dt.bfloat16)
            out_sb = accp.tile([1, C], mybir.dt.float32)
            ps0 = psp.tile([1, HALF], mybir.dt.float32)
            ps1 = psp.tile([1, C - HALF], mybir.dt.float32)

            # dma_starts are software-pipelined two groups ahead; the exp
            # bias (-e) is computed on the otherwise-idle GPSIMD engine so
            # ACT never waits behind the DVE's big streaming ops for it.
            starts = []
            n0 = 0
            for gsz in PLAN:
                starts.append(n0)
                n0 += gsz

            def issue_load(g):
                gsz, n0 = PLAN[g], starts[g]
                xt = xin.tile([P, G_MAX, C1], mybir.dt.float32, tag="xt")
                nc.sync.dma_start(
                    out=xt[:, 0:gsz, :], in_=x_r[:, n0 : n0 + gsz, :]
                )
                neg_e = nep.tile([P, G_MAX], mybir.dt.float32, tag="ne")
                nc.gpsimd.tensor_scalar_mul(
                    neg_e[:, 0:gsz], xt[:, 0:gsz, C], -1.0
                )
                return xt, neg_e

            DEPTH = 3
            window = [issue_load(g) for g in range(DEPTH)]
            for g, gsz in enumerate(PLAN):
                n0 = starts[g]
                xt, neg_e = window.pop(0)
                if g + DEPTH < len(PLAN):
                    window.append(issue_load(g + DEPTH))

                aa = apool.tile([P, G_MAX, C], mybir.dt.bfloat16, tag="aa")
                for j in range(gsz):
                    n = n0 + j
                    nc.scalar.activation(
                        out=aa[:, j, :],
                        in_=xt[:, j, 0:C],
                        func=mybir.ActivationFunctionType.Exp,
                        bias=neg_e[:, j : j + 1],
                        scale=1.0,
                        accum_out=T[:, n : n + 1],
                    )

                aa_f = aa[:, 0:gsz, :].rearrange("p g c -> p (g c)")
                yy = ypool.tile([P, G_MAX, C], mybir.dt.bfloat16, tag="yy")
                yy_f = yy[:, 0:gsz, :].rearrange("p g c -> p (g c)")
                nc.vector.tensor_scalar_add(yy_f, aa_f, 1.0)

                with nc.allow_low_precision(reason="bf16 1/T weights; error averages out over 64k rows"):
                    nc.vector.reciprocal(
                        iT[:, n0 : n0 + gsz], T[:, n0 : n0 + gsz]
                    )

                ww = wpool.tile([P, G_MAX, C], mybir.dt.bfloat16, tag="ww")
                ww_f = ww[:, 0:gsz, :].rearrange("p g c -> p (g c)")
                nc.vector.tensor_scalar(
                    out=ww_f,
                    in0=yy_f.bitcast(mybir.dt.uint16),
                    scalar1=BITLOG_K0,
                    scalar2=BITLOG_S,
                    op0=mybir.AluOpType.subtract,
                    op1=mybir.AluOpType.mult,
                )

                pw = pwpool.tile([P, G_MAX, C], mybir.dt.bfloat16, tag="pw")
                pw_f = pw[:, 0:gsz, :].rearrange("p g c -> p (g c)")
                nc.vector.tensor_tensor(
                    out=pw_f, in0=aa_f, in1=ww_f, op=mybir.AluOpType.mult
                )

                for j in range(gsz):
                    n = n0 + j
                    first, last = n == 0, n == N_BLOCKS - 1
                    nc.tensor.matmul(
                        ps0, iT[:, n : n + 1], pw[:, j, 0:HALF],
                        start=first, stop=last,
                    )
                    nc.tensor.matmul(
                        ps1, iT[:, n : n + 1], pw[:, j, HALF:C],
                        start=first, stop=last,
                    )

            nc.scalar.copy(out_sb[:, 0:HALF], ps0)
            nc.vector.tensor_copy(out_sb[:, HALF:C], ps1)
            nc.sync.dma_start(out=o, in_=out_sb)

    nc.finalize()
    _nc_cache = nc
    return nc


LAST_RESULTS = None


def kernel(input: np.ndarray, target: np.ndarray | None = None, _trace: bool = False, **_unused) -> np.ndarray:
    global LAST_RESULTS
    input = np.ascontiguousarray(np.asarray(input, dtype=np.float32))
    assert input.shape == (B_FULL, C1), input.shape

    nc = _build()
    in_maps = [
        {"x": input[i * B_SHARD : (i + 1) * B_SHARD]} for i in range(N_CORES)
    ]
    res = bass_utils.run_bass_kernel_spmd(
        nc, in_maps, core_ids=list(range(N_CORES)), trace=_trace
    )
    LAST_RESULTS = res
    total = np.float64(0.0)
    for r in res.results:
        total += np.asarray(r["o"], dtype=np.float64).sum()
    loss = total / B_FULL
    return np.float32(loss)


# revision 13
# speedup vs baseline: 1.1438x; 1.0103x over previous
"""Trainium2 Bass kernel for nn_DiscAdvLossForTarget_min (v9).

Math: loss = (1/B) * sum_b V_b/T_b with a = exp(x - e), w = log1p(a),
V = sum_i a*w, T = sum_i a.

The per-row reduction V moves off the DVE onto the (otherwise idle)
tensor engine: scaling rows by 1/T_b turns sum_b V_b/T_b into a full
sum, which PE does as a matmul with the per-block invT column as the
stationary weight vector:

  psum[f] += sum_p invT[p] * pw[p, f],   pw = a * w
  w = (bits(1+a) - K0) * S               (bit-log log1p, DVE 4x)

Blocks are processed in GROUPS (PLAN, mostly 4 blocks) so that
 - each group's input DMA is one dma_start with 16KB contiguous
   per-partition lines (row = p*64 + n layout), which is what keeps the
   aggregate DMA at ~330 GB/s (v8's per-block 4KB lines cost +13us);
 - the three big DVE ops (a+1, bit-log, a*w) each run once per group at
   FD=4000, amortizing the ~120ns per-op SBUF bubble 4x;
 - PE matmuls come in bursts of 8, keeping the PE p-state warmer.
Groups taper to 1-2 blocks at both ends: short first-DMA latency at
start, short exp->mm drain at the end.

Engine budget/core: DMA ~100us (32.8MB at ~330GB/s = the floor),
ACT ~89us (64 exp+accum, nothing else), DVE ~78us, PE ~60us.

Host: loss = (sum of per-class psums over cores) / B.
"""

import numpy as np

import concourse.bacc as bacc
import concourse.bass as bass
import concourse.tile as tile
from concourse import bass_utils, mybir

N_CORES = 8
B_FULL = 65536
C1 = 1001
C = 1000
P = 128
B_SHARD = B_FULL // N_CORES  # 8192
N_BLOCKS = B_SHARD // P  # 64
G_MAX = 4

# bit-log fit: w ~= (bits(y) - K0) * S, a-weighted LS vs log1p
BITLOG_S = 0.00541268
BITLOG_K0 = 16248.447

HALF = 500  # psum bank holds 512 fp32; split the 1000 classes in two

PLAN = [1, 1, 2] + [4] * 14 + [2, 1, 1]
assert sum(PLAN) == N_BLOCKS

_nc_cache = None


def _build() -> bass.Bass:
    global _nc_cache
    if _nc_cache is not None:
        return _nc_cache

    nc = bacc.Bacc("TRN2", debug=False)
    x = nc.dram_tensor("x", [B_SHARD, C1], mybir.dt.float32, kind="ExternalInput").ap()
    o = nc.dram_tensor("o", [1, C], mybir.dt.float32, kind="ExternalOutput").ap()

    # row = p*64 + n: group DMAs get gsz*4004B contiguous per partition
    x_r = x.rearrange("(p n) m -> p n m", p=P, n=N_BLOCKS)

    with tile.TileContext(nc) as tc:
        with (
            tc.tile_pool(name="xin", bufs=5) as xin,
            tc.tile_pool(name="apool", bufs=5) as apool,
            tc.tile_pool(name="ypool", bufs=2) as ypool,
            tc.tile_pool(name="wpool", bufs=2) as wpool,
            tc.tile_pool(name="pwpool", bufs=3) as pwpool,
            tc.tile_pool(name="nep", bufs=4) as nep,
            tc.tile_pool(name="accp", bufs=1) as accp,
            tc.tile_pool(name="psp", bufs=1, space="PSUM") as psp,
        ):
            T = accp.tile([P, N_BLOCKS], mybir.dt.float32)
            iT = accp.tile([P, N_BLOCKS], mybir.dt.bfloat16)
            out_sb = accp.tile([1, C], mybir.dt.float32)
            ps0 = psp.tile([1, HALF], mybir.dt.float32)
            ps1 = psp.tile([1, C - HALF], mybir.dt.float32)

            # dma_starts are software-pipelined two groups ahead; the exp
            # bias (-e) is computed on the otherwise-idle GPSIMD engine so
            # ACT never waits behind the DVE's big streaming ops for it.
            starts = []
            n0 = 0
            for gsz in PLAN:
                starts.append(n0)
                n0 += gsz

            def issue_load(g):
                gsz, n0 = PLAN[g], starts[g]
                xt = xin.tile([P, G_MAX, C1], mybir.dt.float32, tag="xt")
                nc.sync.dma_start(
                    out=xt[:, 0:gsz, :], in_=x_r[:, n0 : n0 + gsz, :]
                )
                neg_e = nep.tile([P, G_MAX], mybir.dt.float32, tag="ne")
                nc.gpsimd.tensor_scalar_mul(
                    neg_e[:, 0:gsz], xt[:, 0:gsz, C], -1.0
                )
                return xt, neg_e

            DEPTH = 3
            window = [issue_load(g) for g in range(DEPTH)]
            for g, gsz in enumerate(PLAN):
                n0 = starts[g]
                xt, neg_e = window.pop(0)
                if g + DEPTH < len(PLAN):
                    window.append(issue_load(g + DEPTH))

                aa = apool.tile([P, G_MAX, C], mybir.dt.bfloat16, tag="aa")
                for j in range(gsz):
                    n = n0 + j
                    nc.scalar.activation(
                        out=aa[:, j, :],
                        in_=xt[:, j, 0:C],
                        func=mybir.ActivationFunctionType.Exp,
                        bias=neg_e[:, j : j + 1],
                        scale=1.0,
                        accum_out=T[:, n : n + 1],
                    )

                aa_f = aa[:, 0:gsz, :].rearrange("p g c -> p (g c)")
                yy = ypool.tile([P, G_MAX, C], mybir.dt.bfloat16, tag="yy")
                yy_f = yy[:, 0:gsz, :].rearrange("p g c -> p (g c)")
                nc.vector.tensor_scalar_add(yy_f, aa_f, 1.0)

                with nc.allow_low_precision(reason="bf16 1/T weights; error averages out over 64k rows"):
                    nc.vector.reciprocal(
                        iT[:, n0 : n0 + gsz], T[:, n0 : n0 + gsz]
                    )

                ww = wpool.tile([P, G_MAX, C], mybir.dt.bfloat16, tag="ww")
                ww_f = ww[:, 0:gsz, :].rearrange("p g c -> p (g c)")
                nc.vector.tensor_scalar(
                    out=ww_f,
                    in0=yy_f.bitcast(mybir.dt.uint16),
                    scalar1=BITLOG_K0,
                    scalar2=BITLOG_S,
                    op0=mybir.AluOpType.subtract,
                    op1=mybir.AluOpType.mult,
                )

                pw = pwpool.tile([P, G_MAX, C], mybir.dt.bfloat16, tag="pw")
                pw_f = pw[:, 0:gsz, :].rearrange("p g c -> p (g c)")
                nc.vector.tensor_tensor(
                    out=pw_f, in0=aa_f, in1=ww_f, op=mybir.AluOpType.mult
                )

                for j in range(gsz):
                    n = n0 + j
                    first, last = n == 0, n == N_BLOCKS - 1
                    nc.tensor.matmul(
                        ps0, iT[:, n : n + 1], pw[:, j, 0:HALF],
                        start=first, stop=last,
                    )
                    nc.tensor.matmul(
                        ps1, iT[:, n : n + 1], pw[:, j, HALF:C],
                        start=first, stop=last,
                    )

            # both copies on ACT: it is idle after its last exp, while the
            # DVE is still finishing the tail groups' stream ops
            nc.scalar.copy(out_sb[:, 0:HALF], ps0)
            nc.scalar.copy(out_sb[:, HALF:C], ps1)
            nc.sync.dma_start(out=o, in_=out_sb)

    nc.finalize()
    _nc_cache = nc
    return nc


LAST_RESULTS = None


def kernel(input: np.ndarray, target: np.ndarray | None = None, _trace: bool = False, **_unused) -> np.ndarray:
    global LAST_RESULTS
    input = np.ascontiguousarray(np.asarray(input, dtype=np.float32))
    assert input.shape == (B_FULL, C1), input.shape

    nc = _build()
    in_maps = [
        {"x": input[i * B_SHARD : (i + 1) * B_SHARD]} for i in range(N_CORES)
    ]
    res = bass_utils.run_bass_kernel_spmd(
        nc, in_maps, core_ids=list(range(N_CORES)), trace=_trace
    )
    LAST_RESULTS = res
    total = np.float64(0.0)
    for r in res.results:
        total += np.asarray(r["o"], dtype=np.float64).sum()
    loss = total / B_FULL
    return np.float32(loss)


# revision 14
# speedup vs baseline: 1.1447x; 1.0007x over previous
"""Trainium2 Bass kernel for nn_DiscAdvLossForTarget_min (v9).

Math: loss = (1/B) * sum_b V_b/T_b with a = exp(x - e), w = log1p(a),
V = sum_i a*w, T = sum_i a.

The per-row reduction V moves off the DVE onto the (otherwise idle)
tensor engine: scaling rows by 1/T_b turns sum_b V_b/T_b into a full
sum, which PE does as a matmul with the per-block invT column as the
stationary weight vector:

  psum[f] += sum_p invT[p] * pw[p, f],   pw = a * w
  w = (bits(1+a) - K0) * S               (bit-log log1p, DVE 4x)

Blocks are processed in GROUPS (PLAN, mostly 4 blocks) so that
 - each group's input DMA is one dma_start with 16KB contiguous
   per-partition lines (row = p*64 + n layout), which is what keeps the
   aggregate DMA at ~330 GB/s (v8's per-block 4KB lines cost +13us);
 - the three big DVE ops (a+1, bit-log, a*w) each run once per group at
   FD=4000, amortizing the ~120ns per-op SBUF bubble 4x;
 - PE matmuls come in bursts of 8, keeping the PE p-state warmer.
Groups taper to 1-2 blocks at both ends: short first-DMA latency at
start, short exp->mm drain at the end.

Engine budget/core: DMA ~100us (32.8MB at ~330GB/s = the floor),
ACT ~89us (64 exp+accum, nothing else), DVE ~78us, PE ~60us.

Host: loss = (sum of per-class psums over cores) / B.
"""

import numpy as np

import concourse.bacc as bacc
import concourse.bass as bass
import concourse.tile as tile
from concourse import bass_utils, mybir

N_CORES = 8
B_FULL = 65536
C1 = 1001
C = 1000
P = 128
B_SHARD = B_FULL // N_CORES  # 8192
N_BLOCKS = B_SHARD // P  # 64
G_MAX = 4

# bit-log fit: w ~= (bits(y) - K0) * S, a-weighted LS vs log1p
BITLOG_S = 0.00541268
BITLOG_K0 = 16248.447

HALF = 500  # psum bank holds 512 fp32; split the 1000 classes in two

PLAN = [1, 1, 2] + [4] * 14 + [2, 1, 1]
assert sum(PLAN) == N_BLOCKS

_nc_cache = None


def _build() -> bass.Bass:
    global _nc_cache
    if _nc_cache is not None:
        return _nc_cache

    nc = bacc.Bacc("TRN2", debug=False)
    x = nc.dram_tensor("x", [B_SHARD, C1], mybir.dt.float32, kind="ExternalInput").ap()
    o = nc.dram_tensor("o", [1, C], mybir.dt.float32, kind="ExternalOutput").ap()

    # row = p*64 + n: group DMAs get gsz*4004B contiguous per partition
    x_r = x.rearrange("(p n) m -> p n m", p=P, n=N_BLOCKS)

    with tile.TileContext(nc) as tc:
        with (
            tc.tile_pool(name="xin", bufs=5) as xin,
            tc.tile_pool(name="apool", bufs=5) as apool,
            tc.tile_pool(name="ypool", bufs=2) as ypool,
            tc.tile_pool(name="wpool", bufs=2) as wpool,
            tc.tile_pool(name="pwpool", bufs=3) as pwpool,
            tc.tile_pool(name="nep", bufs=4) as nep,
            tc.tile_pool(name="accp", bufs=1) as accp,
            tc.tile_pool(name="psp", bufs=1, space="PSUM") as psp,
        ):
            T = accp.tile([P, N_BLOCKS], mybir.dt.float32)
            iT = accp.tile([P, N_BLOCKS], mybir.dt.bfloat16)
            out_sb = accp.tile([1, C], mybir.dt.float32)
            ps0 = psp.tile([1, HALF], mybir.dt.float32)
            ps1 = psp.tile([1, C - HALF], mybir.dt.float32)

            # dma_starts are software-pipelined two groups ahead; the exp
            # bias (-e) is computed on the otherwise-idle GPSIMD engine so
            # ACT never waits behind the DVE's big streaming ops for it.
            starts = []
            n0 = 0
            for gsz in PLAN:
                starts.append(n0)
                n0 += gsz

            def issue_load(g):
                gsz, n0 = PLAN[g], starts[g]
                xt = xin.tile([P, G_MAX, C1], mybir.dt.float32, tag="xt")
                nc.sync.dma_start(
                    out=xt[:, 0:gsz, :], in_=x_r[:, n0 : n0 + gsz, :]
                )
                neg_e = nep.tile([P, G_MAX], mybir.dt.float32, tag="ne")
                nc.gpsimd.tensor_scalar_mul(
                    neg_e[:, 0:gsz], xt[:, 0:gsz, C], -1.0
                )
                return xt, neg_e

            DEPTH = 4
            window = [issue_load(g) for g in range(DEPTH)]
            for g, gsz in enumerate(PLAN):
                n0 = starts[g]
                xt, neg_e = window.pop(0)
                if g + DEPTH < len(PLAN):
                    window.append(issue_load(g + DEPTH))

                aa = apool.tile([P, G_MAX, C], mybir.dt.bfloat16, tag="aa")
                for j in range(gsz):
                    n = n0 + j
                    nc.scalar.activation(
                        out=aa[:, j, :],
                        in_=xt[:, j, 0:C],
                        func=mybir.ActivationFunctionType.Exp,
                        bias=neg_e[:, j : j + 1],
                        scale=1.0,
                        accum_out=T[:, n : n + 1],
                    )

                aa_f = aa[:, 0:gsz, :].rearrange("p g c -> p (g c)")
                yy = ypool.tile([P, G_MAX, C], mybir.dt.bfloat16, tag="yy")
                yy_f = yy[:, 0:gsz, :].rearrange("p g c -> p (g c)")
                nc.vector.tensor_scalar_add(yy_f, aa_f, 1.0)

                with nc.allow_low_precision(reason="bf16 1/T weights; error averages out over 64k rows"):
                    nc.vector.reciprocal(
                        iT[:, n0 : n0 + gsz], T[:, n0 : n0 + gsz]
                    )

                ww = wpool.tile([P, G_MAX, C], mybir.dt.bfloat16, tag="ww")
                ww_f = ww[:, 0:gsz, :].rearrange("p g c -> p (g c)")
                nc.vector.tensor_scalar(
                    out=ww_f,
                    in0=yy_f.bitcast(mybir.dt.uint16),
                    scalar1=BITLOG_K0,
                    scalar2=BITLOG_S,
                    op0=mybir.AluOpType.subtract,
                    op1=mybir.AluOpType.mult,
                )

                pw = pwpool.tile([P, G_MAX, C], mybir.dt.bfloat16, tag="pw")
                pw_f = pw[:, 0:gsz, :].rearrange("p g c -> p (g c)")
                nc.vector.tensor_tensor(
                    out=pw_f, in0=aa_f, in1=ww_f, op=mybir.AluOpType.mult
                )

                for j in range(gsz):
                    n = n0 + j
                    first, last = n == 0, n == N_BLOCKS - 1
                    nc.tensor.matmul(
                        ps0, iT[:, n : n + 1], pw[:, j, 0:HALF],
                        start=first, stop=last,
                    )
                    nc.tensor.matmul(
                        ps1, iT[:, n : n + 1], pw[:, j, HALF:C],
                        start=first, stop=last,
                    )

            # both copies on ACT: it is idle after its last exp, while the
            # DVE is still finishing the tail groups' stream ops
            nc.scalar.copy(out_sb[:, 0:HALF], ps0)
            nc.scalar.copy(out_sb[:, HALF:C], ps1)
            nc.sync.dma_start(out=o, in_=out_sb)

    nc.finalize()
    _nc_cache = nc
    return nc


LAST_RESULTS = None


def kernel(input: np.ndarray, target: np.ndarray | None = None, _trace: bool = False, **_unused) -> np.ndarray:
    global LAST_RESULTS
    input = np.ascontiguousarray(np.asarray(input, dtype=np.float32))
    assert input.shape == (B_FULL, C1), input.shape

    nc = _build()
    in_maps = [
        {"x": input[i * B_SHARD : (i + 1) * B_SHARD]} for i in range(N_CORES)
    ]
    res = bass_utils.run_bass_kernel_spmd(
        nc, in_maps, core_ids=list(range(N_CORES)), trace=_trace
    )
    LAST_RESULTS = res
    total = np.float64(0.0)
    for r in res.results:
        total += np.asarray(r["o"], dtype=np.float64).sum()
    loss = total / B_FULL
    return np.float32(loss)
